# revision 29
# baseline (speedup 1.0000x reference)
"""Trainium2 Bass kernel for nn_DozatLstmCell (ragged LSTM with recurrent dropout).

Strategy (hardcoded, self-contained):
  - Data-parallel: batch B=64 sharded as 8 contiguous sequences per NeuronCore.
  - Feature-major ("transposed") on-chip layout: state tiles are [128, mc, b]
    with p = feature%128, mc = feature//128, b = local batch 0..7.
  - Phase 1: input projection P = x @ Wi.T + (bi+bh) for all T upfront
    (PE-efficient big matmul), kept resident in SBUF as bf16 [128, 16, T*8].
  - Phase 2: 512 sequential LSTM steps; per step 64 accumulating matmuls
    (bf16 weights stationary, h streams) on top of a PSUM tile preloaded
    with P_t, then sigmoid/tanh/lrelu gates and the c/h update.
  - No on-device masking: inactive (t >= len) steps run freely; the host
    zeroes padding in the output and gathers h_final/c_final from the
    output/c histories at t = len-1 (active rows are bitwise unaffected
    by dropping the freeze, since batch rows are independent).
"""

import numpy as np
import ml_dtypes

BF16NP = ml_dtypes.bfloat16

B, T, D, H = 64, 512, 512, 512
NCORES = 8
BC = B // NCORES          # 8 sequences per core
KC = D // 128             # 4 contraction chunks
MC = H // 128             # 4 hidden chunks
GM = 4 * H // 128         # 16 gate-output chunks (gate-major: gm = gate*4 + mc)
LRELU_ALPHA = 0.01        # jax.nn.leaky_relu default

_CACHE = {}
_LAST_RES = None


def _build(t_steps):
    from contextlib import ExitStack
    import concourse.bass as bass  # noqa: F401
    import concourse.tile as tile
    from concourse import bacc, mybir

    F32 = mybir.dt.float32
    BF = mybir.dt.bfloat16
    AF = mybir.ActivationFunctionType

    nc = bacc.Bacc(None, target_bir_lowering=False)

    n_free = t_steps * BC
    xT_d = nc.dram_tensor("xT", [KC, 128, n_free], BF, kind="ExternalInput")
    wiT_d = nc.dram_tensor("wiT", [GM, KC, 128, 128], BF, kind="ExternalInput")
    whT_d = nc.dram_tensor("whT", [GM, KC, 128, 128], BF, kind="ExternalInput")
    bias_d = nc.dram_tensor("biasT", [128, GM], F32, kind="ExternalInput")
    drop_d = nc.dram_tensor("dropT", [128, MC, BC], F32, kind="ExternalInput")
    outT_d = nc.dram_tensor("outT", [t_steps, 128, MC, BC], BF, kind="ExternalOutput")
    cT_d = nc.dram_tensor("cT", [t_steps, 128, MC, BC], F32, kind="ExternalOutput")

    cs = min(512, n_free)  # phase-1 psum column chunk
    nb_chunks = n_free // cs

    with tile.TileContext(nc) as tc, ExitStack() as ctx:
        ppool = ctx.enter_context(tc.tile_pool(name="pP", bufs=1))
        cpool = ctx.enter_context(tc.tile_pool(name="pconst", bufs=1))

        P = ppool.tile([128, GM, n_free], BF, tag="P")
        biasT = cpool.tile([128, GM], F32, tag="biasT")
        nc.sync.dma_start(biasT[:], bias_d[:])
        drop = cpool.tile([128, MC, BC], F32, tag="drop")
        nc.sync.dma_start(drop[:], drop_d[:])

        # ---------------- Phase 1: P = x @ Wi.T + bias ----------------
        with (
            tc.tile_pool(name="pld", bufs=1) as ld_pool,
            tc.tile_pool(name="ps1", bufs=4, space="PSUM") as ps1,
        ):
            x_sb = ld_pool.tile([128, KC, n_free], BF, tag="x")
            for kc in range(KC):
                nc.sync.dma_start(x_sb[:, kc, :], xT_d[kc])
            wi_sb = ld_pool.tile([128, GM, KC, 128], BF, tag="wi")
            for gm in range(GM):
                for kc in range(KC):
                    nc.sync.dma_start(wi_sb[:, gm, kc, :], wiT_d[gm, kc])

            for gm in range(GM):
                for nb in range(nb_chunks):
                    pt = ps1.tile([128, cs], F32, tag="pt")
                    for kc in range(KC):
                        nc.tensor.matmul(
                            pt[:],
                            wi_sb[:, gm, kc, :],
                            x_sb[:, kc, nb * cs:(nb + 1) * cs],
                            start=(kc == 0),
                            stop=(kc == KC - 1),
                        )
                    # P = identity(psum + bias_gm), cast to bf16
                    nc.scalar.activation(
                        P[:, gm, nb * cs:(nb + 1) * cs], pt[:],
                        AF.Identity, bias=biasT[:, gm:gm + 1],
                    )

        # ---------------- Phase 2: sequential scan ----------------
        with (
            tc.tile_pool(name="pwh", bufs=1) as wh_pool,
            tc.tile_pool(name="pstate", bufs=1) as st_pool,
            tc.tile_pool(name="pscr", bufs=4) as scr,
            tc.tile_pool(name="ps2", bufs=4, space="PSUM") as ps2,
        ):
            wh_sb = wh_pool.tile([128, GM, KC, 128], BF, tag="wh")
            for gm in range(GM):
                for kc in range(KC):
                    nc.sync.dma_start(wh_sb[:, gm, kc, :], whT_d[gm, kc])

            h_tiles = [st_pool.tile([128, MC, BC], BF, name=f"h{i}", tag=f"h{i}")
                       for i in range(2)]
            c_tiles = [st_pool.tile([128, MC, BC], F32, name=f"c{i}", tag=f"c{i}")
                       for i in range(2)]
            nc.gpsimd.memset(h_tiles[0][:], 0.0)
            nc.gpsimd.memset(c_tiles[0][:], 0.0)

            import os as _os
            _abl = _os.environ.get("K_ABLATE", "")
            for t in range(t_steps):
                hc = h_tiles[t % 2]
                hn = h_tiles[(t + 1) % 2]
                cc = c_tiles[t % 2]
                cn = c_tiles[(t + 1) % 2]

                pj = ps2.tile([128, GM, BC], F32, tag="pj")
                # preload PSUM with P_t (on ACT); matmuls accumulate on top
                nc.scalar.activation(pj[:], P[:, :, t * BC:(t + 1) * BC], AF.Copy)
                if _abl != "nomm":
                    # gm blocks (host-permuted): 0:4 = i, 4:8 = f, 8:12 = o,
                    # 12:16 = g.  Emit the tanh-gate matmuls first so Tanh
                    # can start while the sigmoid gates are still on the PE.
                    for gm in (12, 13, 14, 15, 0, 1, 2, 3, 4, 5, 6, 7, 8, 9, 10, 11):
                        for kc in range(KC):
                            nc.tensor.matmul(
                                pj[:, gm, :],
                                wh_sb[:, gm, kc, :],
                                hc[:, kc, :],
                                start=False,
                                stop=(kc == KC - 1),
                                skip_group_check=True,
                            )

                g_sb = scr.tile([128, GM, BC], F32, tag="g")
                nc.scalar.activation(g_sb[:, 12:16, :], pj[:, 12:16, :], AF.Tanh)
                nc.scalar.activation(g_sb[:, 0:12, :], pj[:, 0:12, :], AF.Sigmoid)

                t2 = scr.tile([128, MC, BC], F32, tag="t2")
                nc.vector.tensor_mul(t2[:], g_sb[:, 4:8, :], cc[:])
                t1 = scr.tile([128, MC, BC], F32, tag="t1")
                nc.vector.tensor_mul(t1[:], g_sb[:, 0:4, :], g_sb[:, 12:16, :])
                nc.vector.tensor_add(cn[:], t1[:], t2[:])

                # h = od*leaky_relu(c) = max(0.01*(od*c), od*c) since od >= 0
                od = scr.tile([128, MC, BC], F32, tag="od")
                nc.vector.tensor_mul(od[:], g_sb[:, 8:12, :], drop[:])
                a_br = scr.tile([128, MC, BC], F32, tag="a_br")
                nc.vector.tensor_mul(a_br[:], od[:], cn[:])
                nc.vector.scalar_tensor_tensor(
                    hn[:], a_br[:], LRELU_ALPHA, a_br[:],
                    op0=mybir.AluOpType.mult, op1=mybir.AluOpType.max)

                if _abl != "nodma":
                    nc.sync.dma_start(outT_d[t], hn[:])
                    nc.sync.dma_start(cT_d[t], cn[:])

    nc.compile()
    return nc


def _get_nc(t_steps=T):
    if t_steps not in _CACHE:
        _CACHE[t_steps] = _build(t_steps)
    return _CACHE[t_steps]


def kernel(x, lengths, dropout_mask, Wi, bi, Wh, bh, t_steps=T):
    from concourse.bass_utils import run_bass_kernel_spmd

    x = np.asarray(x, dtype=np.float32)
    lengths = np.asarray(lengths, dtype=np.int32)
    dropout_mask = np.asarray(dropout_mask, dtype=np.float32)
    Wi = np.asarray(Wi, dtype=np.float32)
    bi = np.asarray(bi, dtype=np.float32)
    Wh = np.asarray(Wh, dtype=np.float32)
    bh = np.asarray(bh, dtype=np.float32)

    nc = _get_nc(t_steps)

    # permute gate blocks (i, f, g, o) -> (i, f, o, g) so all sigmoid gates
    # are contiguous on-device
    perm = np.r_[0:H, H:2 * H, 3 * H:4 * H, 2 * H:3 * H]
    Wi_p, Wh_p, bias_p = Wi[perm], Wh[perm], (bi + bh)[perm]
    wiT = np.ascontiguousarray(
        Wi_p.reshape(GM, 128, KC, 128).transpose(0, 2, 3, 1)).astype(BF16NP)
    whT = np.ascontiguousarray(
        Wh_p.reshape(GM, 128, KC, 128).transpose(0, 2, 3, 1)).astype(BF16NP)
    biasT = np.ascontiguousarray(bias_p.reshape(GM, 128).T).astype(np.float32)

    in_maps = []
    for c in range(NCORES):
        xc = x[c * BC:(c + 1) * BC, :t_steps]          # [BC, t, D]
        xT = np.ascontiguousarray(
            xc.reshape(BC, t_steps, KC, 128).transpose(2, 3, 1, 0)
        ).reshape(KC, 128, t_steps * BC).astype(BF16NP)
        dmc = dropout_mask[c * BC:(c + 1) * BC]        # [BC, H]
        dropT = np.ascontiguousarray(
            dmc.reshape(BC, MC, 128).transpose(2, 1, 0)).astype(np.float32)
        in_maps.append({"xT": xT, "wiT": wiT, "whT": whT,
                        "biasT": biasT, "dropT": dropT})

    res = run_bass_kernel_spmd(nc, in_maps, core_ids=list(range(NCORES)))
    global _LAST_RES
    _LAST_RES = res

    outputs = np.zeros((B, t_steps, H), dtype=np.float32)
    h_final = np.zeros((B, H), dtype=np.float32)
    c_final = np.zeros((B, H), dtype=np.float32)
    for c in range(NCORES):
        o = res.results[c]["outT"].astype(np.float32)   # [t,128,MC,BC]
        ch = res.results[c]["cT"]                       # [t,128,MC,BC] f32
        o = o.transpose(3, 0, 2, 1).reshape(BC, t_steps, H)
        ch = ch.transpose(3, 0, 2, 1).reshape(BC, t_steps, H)
        for b in range(BC):
            gb = c * BC + b
            L = min(int(lengths[gb]), t_steps)
            h_final[gb] = o[b, L - 1]
            c_final[gb] = ch[b, L - 1]
            outputs[gb, :L] = o[b, :L]
    return outputs, h_final[None], c_final[None]
